# revision 28
# baseline (speedup 1.0000x reference)
"""DeeperGCN Trainium2 kernel (8 NeuronCores, SPMD) — quarter-pipelined v2.

Strategy:
  - Nodes padded to NPAD = 8*NSH, sharded by dst across 8 cores; each
    core's nodes split into 4 quarters of NQ.
  - Per conv layer the per-node message tables u = m*exp(t*m), w =
    exp(t*m) (m = relu(h)+eps) are AllGathered in FOUR quarter pieces
    (one collective per (conv, quarter), double-buffered across convs)
    so edge gathers can start as soon as a piece lands.
  - Edge aggregation: per dst-quarter, 3-4 window groups; per group one
    dma_gather per src-quarter (4 SWDGE queues), indices sorted by
    source row within each (window, src-quarter) bucket for HBM
    locality, then segmented-sum over dst via TensorE matmuls with
    one-hot fp8 R matrices (host-precomputed, one streamed DMA/group).
  - The conv is pipelined at quarter granularity: after the groups of
    dst-quarter q finish, the MLP (M phase), next conv's U phase and
    its AllGather trigger for quarter q are interleaved with the
    gathers of quarter q+1, so collectives/MLP hide under gather DMA.
  - MLP+LN evaluated feature-major with LN mean-centering and biases
    folded into the weights host-side (ones-row trick), variance via
    ones-matmul of the square, rstd via ACT Abs_reciprocal_sqrt.
Host preprocessing (edge bucketing, R matrices, transpose/cast of x,
weight centering) is numpy; only device NEFF time counts.
"""
import numpy as np
import ml_dtypes
from dataclasses import dataclass

EPS_MSG = 1e-7
LN_EPS = 1e-5
NCORES = 8


@dataclass
class Cfg:
    N: int = 100000
    E: int = 1000000
    F_IN: int = 500
    H: int = 64
    C: int = 3
    L: int = 3
    NSH: int = 12800            # nodes/core, 100 windows of 128
    NSW: int = 4                # src quarters (one AllGather piece each)
    SLOTS_G: int = 104          # gather-buffer chunk slots per group
    MCH: int = 400              # MLP node-chunk (<=512)

    @property
    def NPAD(self):
        return NCORES * self.NSH

    @property
    def NW(self):
        return self.NSH // 128   # 100

    @property
    def NQ(self):
        return self.NSH // 4     # 3200

    @property
    def NWQ(self):
        return self.NW // 4      # 25 windows per quarter

    @property
    def WSZ(self):
        w = self.NPAD // self.NSW   # rows per src window (25600)
        assert w <= 32768
        return w

    @property
    def HH(self):
        return 2 * self.H

    @property
    def FPAD(self):
        return ((self.F_IN + 127) // 128) * 128

    @property
    def CPQ(self):
        return self.NQ // self.MCH   # 8


CFG = Cfg()


def _cdiv(a, b):
    return (a + b - 1) // b


# --------------------------------------------------------------------------
# host: edge structures
# --------------------------------------------------------------------------

def build_edge_structs(edge_index, cfg=CFG):
    src = np.asarray(edge_index[0], np.int64)
    dst = np.asarray(edge_index[1], np.int64)
    NSH, NW, NSW, NQ, NWQ = cfg.NSH, cfg.NW, cfg.NSW, cfg.NQ, cfg.NWQ

    core = dst // NSH
    dstloc = dst % NSH
    w = dstloc // 128
    dcol = dstloc % 128
    score = src // NSH
    sloc = src % NSH
    s = sloc // NQ                      # src quarter (table piece)
    srow = score * NQ + (sloc % NQ)     # row within piece table
    assert srow.max() < 32768

    counts = np.zeros((NCORES, NW, NSW), np.int64)
    np.add.at(counts, (core, w, s), 1)
    PBc = _cdiv(counts, 128).max(axis=0)           # [NW, NSW]
    PBc = np.maximum(PBc, 1)
    win_chunks = PBc.sum(axis=1)                   # [NW]

    # groups: per dst-quarter, balanced split with max group <= SLOTS_G
    groups = []                                    # (q, w0, w1)
    for q in range(4):
        wq = np.arange(q * NWQ, (q + 1) * NWQ)
        tot = int(win_chunks[wq].sum())
        ng = max(1, _cdiv(tot, cfg.SLOTS_G))
        while True:
            cuts = [q * NWQ]
            acc, target, gi = 0, tot / ng, 1
            ok = True
            sizes = []
            for wi in wq:
                acc += int(win_chunks[wi])
                if acc >= target * gi and gi < ng:
                    cuts.append(int(wi) + 1)
                    gi += 1
            cuts.append((q + 1) * NWQ)
            sizes = [int(win_chunks[cuts[i]:cuts[i + 1]].sum())
                     for i in range(len(cuts) - 1)]
            if max(sizes) <= cfg.SLOTS_G:
                break
            ng += 1
        for i in range(len(cuts) - 1):
            if cuts[i] < cuts[i + 1]:
                groups.append((q, cuts[i], cuts[i + 1]))

    # global slot layout: for si (gather phase), for group, for wi,
    # chunks contiguous -- phase si of all groups is one contiguous run
    sg_base = {}         # (gid, si) -> first global slot of that run
    wc_base = {}         # (wi, si) -> global slot base of that bucket
    slot = 0
    for si in range(NSW):
        for gid, (q, w0, w1) in enumerate(groups):
            sg_base[(gid, si)] = slot
            for wi in range(w0, w1):
                wc_base[(wi, si)] = slot
                slot += int(PBc[wi, si])
    CTOT = slot
    IWALL = CTOT * 128
    SLOTS_SI = max(int(PBc[w0:w1, si].sum())
                   for (q, w0, w1) in groups for si in range(NSW))

    # per-core gather indices + R matrices (src-sorted within buckets)
    gidx_all, rmat_all = [], []
    key = (w * NSW + s)
    for k in range(NCORES):
        sel = core == k
        kk, ksrow, kdcol = key[sel], srow[sel], dcol[sel]
        order = np.lexsort((ksrow, kk))
        kk, ksrow, kdcol = kk[order], ksrow[order], kdcol[order]
        bnd = np.searchsorted(kk, np.arange(NW * NSW + 1))
        gidx = np.zeros(IWALL, np.int16)
        rmat = np.zeros((128, CTOT, 128), ml_dtypes.float8_e4m3)
        for wi in range(NW):
            for si in range(NSW):
                a, b = bnd[wi * NSW + si], bnd[wi * NSW + si + 1]
                n = b - a
                cap = int(PBc[wi, si]) * 128
                assert n <= cap, (wi, si, n, cap)
                base = wc_base[(wi, si)] * 128
                gidx[base: base + n] = ksrow[a:b]
                j = np.arange(n)
                rmat[j % 128, wc_base[(wi, si)] + j // 128, kdcol[a:b]] = 1.0
        gidx_all.append(gidx)
        rmat_all.append(rmat)

    return dict(PBc=PBc, groups=groups, sg_base=sg_base,
                wc_base=wc_base, CTOT=CTOT, IWALL=IWALL, SLOTS_SI=SLOTS_SI,
                gidx=gidx_all, rmat=rmat_all)


# --------------------------------------------------------------------------
# device builder
# --------------------------------------------------------------------------

def build_nc(structs, cfg=CFG):
    import concourse.bass as bass
    import concourse.tile as tile
    from concourse import bacc, mybir
    from contextlib import ExitStack

    dt = mybir.dt
    AF = mybir.ActivationFunctionType
    AL = mybir.AluOpType
    PBc, groups = structs["PBc"], structs["groups"]
    sg_base = structs["sg_base"]
    wc_base, CTOT, IWALL = structs["wc_base"], structs["CTOT"], structs["IWALL"]
    NSH, NW, NSW, WSZ = cfg.NSH, cfg.NW, cfg.NSW, cfg.WSZ
    NQ, NWQ = cfg.NQ, cfg.NWQ
    H, HH, C, L, FPAD = cfg.H, cfg.HH, cfg.C, cfg.L, cfg.FPAD
    MCH, CPQ = cfg.MCH, cfg.CPQ
    FCH = FPAD // 128

    PCOL = {"encb": 0, "b2r": 1, "eps": 2, "tiny": 3, "zero": 4}
    nc_col = 5
    for l in range(L):
        for nm in ("g1", "be1", "b2c", "ng", "nb", "t"):
            PCOL[(nm, l)] = nc_col
            nc_col += 1
    NPCOL = nc_col

    nc = bacc.Bacc("TRN2", num_swdge_queues=4, dynamic_dma_scratch_size=32768)
    xt_d = nc.declare_dram_parameter("xt", [128, FCH, NSH], dt.bfloat16, isOutput=False)
    encw_d = nc.declare_dram_parameter("encw", [128, FCH, H], dt.bfloat16, isOutput=False)
    w1_d = nc.declare_dram_parameter("w1", [H + 1, L, HH], dt.bfloat16, isOutput=False)
    w2_d = nc.declare_dram_parameter("w2", [HH, L, H], dt.bfloat16, isOutput=False)
    w2r_d = nc.declare_dram_parameter("w2r", [HH, H], dt.bfloat16, isOutput=False)
    linw_d = nc.declare_dram_parameter("linw", [H + 1, C], dt.bfloat16, isOutput=False)
    ident_d = nc.declare_dram_parameter("ident", [128, 128], dt.bfloat16, isOutput=False)
    params_d = nc.declare_dram_parameter("params", [128, NPCOL], dt.float32, isOutput=False)
    gidx_d = nc.declare_dram_parameter("gidx", [128, IWALL // 16], dt.int16, isOutput=False)
    rmat_d = nc.declare_dram_parameter("rmat", [128, CTOT, 128], dt.float8e4, isOutput=False)
    outp_d = nc.declare_dram_parameter("outp", [C, NSH], dt.float32, isOutput=True)

    uvsh = [[nc.dram_tensor(f"uvsh{p}_{q}", [NQ, HH], dt.bfloat16)
             for q in range(4)] for p in range(2)]
    tabs = [[nc.dram_tensor(f"tab{p}_{q}", [WSZ, HH], dt.bfloat16,
                            addr_space="Shared") for q in range(4)]
            for p in range(2)]

    conv_params = [0] + list(range(L))          # [0, 0, 1, 2]

    with tile.TileContext(nc) as tc, ExitStack() as ctx:
        const = ctx.enter_context(tc.tile_pool(name="const", bufs=1))
        sb_par = const.tile([128, NPCOL], dt.float32)
        nc.sync.dma_start(sb_par[:], params_d[:])
        sb_encw = const.tile([128, FCH, H], dt.bfloat16)
        nc.sync.dma_start(sb_encw[:], encw_d[:])
        sb_w1 = const.tile([H + 1, L, HH], dt.bfloat16)
        nc.sync.dma_start(sb_w1[:], w1_d[:])
        sb_w2 = const.tile([HH, L, H], dt.bfloat16)
        nc.sync.dma_start(sb_w2[:], w2_d[:])
        sb_w2r = const.tile([HH, H], dt.bfloat16)
        nc.sync.dma_start(sb_w2r[:], w2r_d[:])
        sb_linw = const.tile([H + 1, C], dt.bfloat16)
        nc.sync.dma_start(sb_linw[:], linw_d[:])
        sb_id = const.tile([128, 128], dt.bfloat16)
        nc.sync.dma_start(sb_id[:], ident_d[:])
        sb_o128 = const.tile([128, 128], dt.bfloat16)
        nc.vector.memset(sb_o128[:], 1.0 / 128)
        sb_o64 = const.tile([H, H], dt.bfloat16)
        nc.vector.memset(sb_o64[:], 1.0 / H)

        def pcol(key, rows=128):
            cidx = PCOL[key]
            return sb_par[0:rows, cidx: cidx + 1]

        master = ctx.enter_context(tc.tile_pool(name="master", bufs=1))
        hT = master.tile([H + 1, NSH], dt.bfloat16)     # row H == 1.0
        rbf65 = master.tile([H + 1, NQ], dt.bfloat16)   # row H == 1.0
        nc.vector.memset(hT[H:H + 1, :], 1.0)
        nc.vector.memset(rbf65[H:H + 1, :], 1.0)

        # persistent pools
        ndq = ctx.enter_context(tc.tile_pool(name="ndq", bufs=4))
        uvq = ctx.enter_context(tc.tile_pool(name="uvq", bufs=1))
        utp = ctx.enter_context(tc.tile_pool(name="utp", bufs=1))
        stp = ctx.enter_context(tc.tile_pool(name="stp", bufs=2))
        t64 = ctx.enter_context(tc.tile_pool(name="t64", bufs=3))
        t128 = ctx.enter_context(tc.tile_pool(name="t128", bufs=3))
        otp = ctx.enter_context(tc.tile_pool(name="otp", bufs=2))
        tpp = ctx.enter_context(tc.tile_pool(name="tpp", bufs=1, space="PSUM"))

        # ------------- emit helpers -------------
        def emit_U(par, q, l):
            """uv table rows for quarter q from hT -> uvsh[par]."""
            slq = slice(q * NQ, (q + 1) * NQ)
            m_t = utp.tile([H, NQ], dt.bfloat16, tag="mt")
            nc.vector.tensor_scalar(m_t[:], hT[0:H, slq], 0.0, EPS_MSG,
                                    AL.max, AL.add)
            wq = utp.tile([H, NQ], dt.bfloat16, tag="wq")
            nc.scalar.activation(wq[:], m_t[:], AF.Exp, scale=pcol(("t", l), H))
            uv = uvq.tile([128, NQ], dt.bfloat16, tag="uv")
            nc.vector.tensor_scalar_add(uv[H:HH, :], wq[:], pcol("zero", H))
            nc.vector.tensor_mul(uv[0:H, :], m_t[:], wq[:])
            TB = 5
            for b in range(NWQ // TB):
                st = stp.tile([128, TB, HH], dt.bfloat16, tag="st")
                for t in range(TB):
                    nt = b * TB + t
                    tp = tpp.tile([128, HH], dt.bfloat16, tag="tp")
                    nc.tensor.transpose(tp[:], uv[:, nt * 128:(nt + 1) * 128],
                                        sb_id[:])
                    if nt % 2 == 0:
                        nc.scalar.copy(st[:, t, :], tp[:])
                    else:
                        nc.vector.tensor_copy(st[:, t, :], tp[:])
                r0 = b * TB * 128
                out_ap = uvsh[par][q][r0: r0 + TB * 128, :]
                nc.sync.dma_start(
                    out_ap.rearrange("(t p) h -> p t h", p=128), st[:])

        def emit_AG(par, si):
            nc.gpsimd.collective_compute(
                "AllGather", mybir.AluOpType.bypass,
                replica_groups=[list(range(NCORES))],
                ins=[uvsh[par][si][:, :]], outs=[tabs[par][si][:, :]])

        def emit_M(conv, l, q, nd):
            """agg -> MLP(+folded LN) -> residual for quarter q."""
            is_first = conv == 0
            slq = slice(q * NQ, (q + 1) * NQ)
            dn_lo = t64.tile([H, NQ], dt.bfloat16, tag="t64")
            nc.vector.tensor_scalar_max(dn_lo[:], nd[H:HH, :], 0.5)
            s_t = t64.tile([H, NQ], dt.bfloat16, tag="t64")
            nc.scalar.activation(s_t[:], dn_lo[:], AF.Abs_reciprocal_sqrt,
                                 bias=pcol("tiny", H))
            rden = t64.tile([H, NQ], dt.bfloat16, tag="t64")
            nc.vector.tensor_mul(rden[:], s_t[:], s_t[:])
            t1 = t64.tile([H, NQ], dt.bfloat16, tag="t64")
            nc.vector.tensor_mul(t1[:], nd[0:H, :], rden[:])
            nc.vector.tensor_add(rbf65[0:H, :], t1[:], hT[0:H, slq])
            ycs = t128.tile([HH, NQ], dt.bfloat16, tag="t128")
            rstd = t128.tile([HH, NQ], dt.bfloat16, tag="t128")
            for c in range(CPQ):
                lo = c * MCH
                ps1 = mpa.tile([HH, MCH], dt.float32, tag="mma")
                nc.tensor.matmul(ps1[:], sb_w1[:, l, :], rbf65[:, lo:lo + MCH])
                nc.vector.tensor_copy(ycs[:, lo:lo + MCH], ps1[:])
            sq = t128.tile([HH, NQ], dt.bfloat16, tag="t128")
            nc.vector.tensor_mul(sq[:], ycs[:], ycs[:])
            for c in range(CPQ):
                lo = c * MCH
                pv = mpa.tile([HH, MCH], dt.float32, tag="mma")
                nc.tensor.matmul(pv[:], sb_o128[:], sq[:, lo:lo + MCH])
                nc.scalar.activation(rstd[:, lo:lo + MCH], pv[:],
                                     AF.Abs_reciprocal_sqrt, bias=pcol("eps"))
            t_t = t128.tile([HH, NQ], dt.bfloat16, tag="t128")
            nc.vector.tensor_mul(t_t[:], ycs[:], rstd[:])
            h1 = t128.tile([HH, NQ], dt.bfloat16, tag="t128")
            nc.scalar.activation(h1[:], t_t[:], AF.Relu,
                                 bias=pcol(("be1", l)), scale=pcol(("g1", l)))
            if is_first:
                for c in range(CPQ):
                    lo = c * MCH
                    sl = slice(q * NQ + lo, q * NQ + lo + MCH)
                    ps2 = mpb.tile([H, MCH], dt.float32, tag="mmb")
                    nc.tensor.matmul(ps2[:], sb_w2r[:, :], h1[:, lo:lo + MCH])
                    nc.vector.tensor_scalar_add(hT[0:H, sl], ps2[:],
                                                pcol("b2r", H))
            else:
                ycs2 = t64.tile([H, NQ], dt.bfloat16, tag="t64")
                rstd2 = t64.tile([H, NQ], dt.bfloat16, tag="t64")
                for c in range(CPQ):
                    lo = c * MCH
                    ps2 = mpb.tile([H, MCH], dt.float32, tag="mmb")
                    nc.tensor.matmul(ps2[:], sb_w2[:, l, :], h1[:, lo:lo + MCH])
                    nc.vector.tensor_scalar_add(ycs2[:, lo:lo + MCH], ps2[:],
                                                pcol(("b2c", l), H))
                sq2 = t64.tile([H, NQ], dt.bfloat16, tag="t64")
                nc.vector.tensor_mul(sq2[:], ycs2[:], ycs2[:])
                for c in range(CPQ):
                    lo = c * MCH
                    pv2 = mpb.tile([H, MCH], dt.float32, tag="mmb")
                    nc.tensor.matmul(pv2[:], sb_o64[:], sq2[:, lo:lo + MCH])
                    nc.scalar.activation(rstd2[:, lo:lo + MCH], pv2[:],
                                         AF.Abs_reciprocal_sqrt,
                                         bias=pcol("eps", H))
                t2 = t64.tile([H, NQ], dt.bfloat16, tag="t64")
                nc.vector.tensor_mul(t2[:], ycs2[:], rstd2[:])
                c_t = t64.tile([H, NQ], dt.bfloat16, tag="t64")
                nc.scalar.activation(c_t[:], t2[:], AF.Relu,
                                     bias=pcol(("nb", l), H),
                                     scale=pcol(("ng", l), H))
                nc.vector.tensor_add(hT[0:H, slq], hT[0:H, slq], c_t[:])

        def emit_head(q):
            slq = slice(q * NQ, (q + 1) * NQ)
            yc = t128.tile([H, NQ], dt.bfloat16, tag="t128")
            rstd = t64.tile([H, NQ], dt.bfloat16, tag="t64")
            for c in range(CPQ):
                lo = c * MCH
                sl = slice(q * NQ + lo, q * NQ + lo + MCH)
                pmu = mpb.tile([H, MCH], dt.float32, tag="mmb")
                nc.tensor.matmul(pmu[:], sb_o64[:], hT[0:H, sl])
                nc.vector.tensor_sub(yc[:, lo:lo + MCH], hT[0:H, sl], pmu[:])
            sq = t64.tile([H, NQ], dt.bfloat16, tag="t64")
            nc.vector.tensor_mul(sq[:], yc[:], yc[:])
            for c in range(CPQ):
                lo = c * MCH
                pv = mpb.tile([H, MCH], dt.float32, tag="mmb")
                nc.tensor.matmul(pv[:], sb_o64[:], sq[:, lo:lo + MCH])
                nc.scalar.activation(rstd[:, lo:lo + MCH], pv[:],
                                     AF.Abs_reciprocal_sqrt, bias=pcol("eps", H))
            t_t = t64.tile([H, NQ], dt.bfloat16, tag="t64")
            nc.vector.tensor_mul(t_t[:], yc[:], rstd[:])
            f65 = t128.tile([H + 1, NQ], dt.bfloat16, tag="t128")
            nc.vector.memset(f65[H:H + 1, :], 1.0)
            nc.scalar.activation(f65[0:H, :], t_t[:], AF.Relu,
                                 bias=pcol(("nb", 0), H),
                                 scale=pcol(("ng", 0), H))
            for c in range(CPQ):
                lo = c * MCH
                sl = slice(q * NQ + lo, q * NQ + lo + MCH)
                pso = mpa.tile([C, MCH], dt.float32, tag="mma")
                nc.tensor.matmul(pso[:], sb_linw[:, :], f65[:, lo:lo + MCH])
                ot = otp.tile([C, MCH], dt.float32, tag="ot")
                nc.vector.tensor_copy(ot[:], pso[:])
                nc.sync.dma_start(outp_d[:, sl], ot[:])

        # ---------------- encoder (pipelined per quarter) ----------------
        with tc.tile_pool(name="enc", bufs=3) as ep, \
             tc.tile_pool(name="encps", bufs=2, space="PSUM") as pp:
            for q in range(4):
                for cc in range(CPQ):
                    sl = slice(q * NQ + cc * MCH, q * NQ + (cc + 1) * MCH)
                    xtile = ep.tile([128, FCH, MCH], dt.bfloat16)
                    nc.sync.dma_start(xtile[:], xt_d[:, :, sl])
                    ps = pp.tile([H, MCH], dt.float32)
                    for fc in range(FCH):
                        nc.tensor.matmul(ps[:], sb_encw[:, fc, :],
                                         xtile[:, fc, :],
                                         start=(fc == 0), stop=(fc == FCH - 1))
                    nc.vector.tensor_scalar_add(hT[0:H, sl], ps[:],
                                                pcol("encb", H))
                emit_U(0, q, conv_params[0])

        # gather/psum pools opened after encoder frees its SBUF
        gp = ctx.enter_context(tc.tile_pool(name="gp", bufs=4))
        rp = ctx.enter_context(tc.tile_pool(name="rp", bufs=4))
        gxp = ctx.enter_context(tc.tile_pool(name="gxp", bufs=4))
        gsp = ctx.enter_context(tc.tile_pool(name="gsp", bufs=3, space="PSUM"))
        mpa = ctx.enter_context(tc.tile_pool(name="mpa", bufs=2, space="PSUM"))
        mpb = ctx.enter_context(tc.tile_pool(name="mpb", bufs=2, space="PSUM"))

        # ---------------- conv layers, src-phase pipelined ----------------
        # Per conv: 4 src phases.  Phase si starts with its piece AllGather
        # trigger (which BLOCKS the Pool engine until the collective lands
        # -- but phases si-1's gathers are already draining on the SDMA
        # queues and the PE keeps consuming them, so the collective hides).
        # Each group's phase-si slots are gathered as one instruction;
        # windows accumulate in PSUM per phase and are added into the
        # quarter's persistent numden tile.  M/U of quarter q run lagged
        # inside phase 3 once q's windows are complete.
        def do_MU(conv, l, q, nd):
            emit_M(conv, l, q, nd)
            if conv + 1 < len(conv_params):
                emit_U(1 - conv % 2, q, conv_params[conv + 1])
            else:
                emit_head(q)

        WT = 3      # windows per PSUM accumulation tile
        for conv, l in enumerate(conv_params):
            par = conv % 2
            ndt = {}
            for si in range(NSW):
                emit_AG(par, si)
                for gid, (q, w0, w1) in enumerate(groups):
                    if q not in ndt:
                        ndt[q] = ndq.tile([HH, NQ], dt.bfloat16, tag="nd",
                                          name=f"nd_{conv}_{q}")
                    nd = ndt[q]
                    nsl = int(PBc[w0:w1, si].sum())
                    base = sg_base[(gid, si)]
                    rtile = rp.tile([128, nsl, 128], dt.float8e4, tag="rt")
                    nc.sync.dma_start(rtile[:],
                                      rmat_d[:, base: base + nsl, :])
                    gx = gxp.tile([128, nsl * 8], dt.int16, tag="gx")
                    nc.sync.dma_start(
                        gx[:], gidx_d[:, base * 8: (base + nsl) * 8])
                    gbuf = gp.tile([128, nsl, HH], dt.bfloat16, tag="gbuf")
                    nc.gpsimd.dma_gather(
                        gbuf[:], tabs[par][si][:, :], gx[:],
                        nsl * 128, nsl * 128, HH, single_packet=False,
                        queue_num=gid % 4)
                    wi = w0
                    while wi < w1:
                        wn = min(WT, w1 - wi)
                        ps = gsp.tile([HH, WT * 128], dt.float32, tag="gs")
                        for k in range(wn):
                            cb = wc_base[(wi + k, si)] - base
                            nchw = int(PBc[wi + k, si])
                            for j in range(nchw):
                                nc.tensor.matmul(
                                    ps[:, k * 128:(k + 1) * 128],
                                    gbuf[:, cb + j, :], rtile[:, cb + j, :],
                                    start=(j == 0), stop=(j == nchw - 1))
                        wloc = wi - q * NWQ
                        wsl = slice(wloc * 128, (wloc + wn) * 128)
                        if si == 0:
                            nc.scalar.copy(nd[:, wsl], ps[:, : wn * 128])
                        else:
                            nc.vector.tensor_add(nd[:, wsl], nd[:, wsl],
                                                 ps[:, : wn * 128])
                        wi += wn
                    if si == NSW - 1:
                        last_of_q = (gid + 1 == len(groups)
                                     or groups[gid + 1][0] != q)
                        if last_of_q and q > 0:
                            do_MU(conv, l, q - 1, ndt[q - 1])
                        if last_of_q and q == 3:
                            do_MU(conv, l, 3, ndt[3])

    nc.compile()
    return nc, NPCOL, PCOL


# --------------------------------------------------------------------------
# host: input packing
# --------------------------------------------------------------------------

def pack_inputs(inputs, structs, NPCOL, PCOL, cfg=CFG):
    bf16 = ml_dtypes.bfloat16
    NSH, NPAD, FPAD = cfg.NSH, cfg.NPAD, cfg.FPAD
    H, HH, C, L = cfg.H, cfg.HH, cfg.C, cfg.L

    x = np.asarray(inputs["x"], np.float32)
    xp = np.zeros((NPAD, FPAD), np.float32)
    xp[: x.shape[0], : x.shape[1]] = x

    encw = np.zeros((FPAD, H), np.float32)
    encw[: cfg.F_IN] = np.asarray(inputs["enc_W"], np.float32)
    encw = np.ascontiguousarray(
        encw.reshape(FPAD // 128, 128, H).transpose(1, 0, 2)).astype(bf16)

    # W1 with LN mean-centering folded in + centered bias as a 65th row
    W1 = np.asarray(inputs["W1"], np.float32)          # [L, H, HH]
    b1 = np.asarray(inputs["b1"], np.float32)          # [L, HH]
    W1c = W1 - W1.mean(axis=2, keepdims=True)
    b1c = b1 - b1.mean(axis=1, keepdims=True)
    w1s = np.concatenate([W1c, b1c[:, None, :]], axis=1)   # [L, H+1, HH]
    w1s = np.ascontiguousarray(w1s.transpose(1, 0, 2)).astype(bf16)

    W2 = np.asarray(inputs["W2"], np.float32)          # [L, HH, H]
    W2c = W2 - W2.mean(axis=2, keepdims=True)
    w2s = np.ascontiguousarray(W2c.transpose(1, 0, 2)).astype(bf16)
    w2r = np.ascontiguousarray(W2[0]).astype(bf16)     # uncentered, conv 0

    linw = np.asarray(inputs["lin_W"], np.float32)
    linb = np.asarray(inputs["lin_b"], np.float32)
    linw65 = np.concatenate([linw, linb[None, :]], axis=0).astype(bf16)
    ident = np.eye(128, dtype=bf16)

    b2 = np.asarray(inputs["b2"], np.float32)          # [L, H]
    b2c = b2 - b2.mean(axis=1, keepdims=True)

    params = np.zeros((128, NPCOL), np.float32)
    params[:H, PCOL["encb"]] = inputs["enc_b"]
    params[:H, PCOL["b2r"]] = b2[0]
    params[:, PCOL["eps"]] = LN_EPS
    params[:, PCOL["tiny"]] = 1e-30
    for l in range(L):
        params[:, PCOL[("g1", l)]] = inputs["g1"][l]
        params[:, PCOL[("be1", l)]] = inputs["be1"][l]
        params[:H, PCOL[("b2c", l)]] = b2c[l]
        params[:H, PCOL[("ng", l)]] = inputs["ng"][l]
        params[:H, PCOL[("nb", l)]] = inputs["nb"][l]
        params[:, PCOL[("t", l)]] = float(np.asarray(inputs["t"][l]))

    in_maps = []
    for k in range(NCORES):
        xs = np.ascontiguousarray(xp[k * NSH:(k + 1) * NSH].T)  # [FPAD, NSH]
        xs = np.ascontiguousarray(
            xs.reshape(FPAD // 128, 128, NSH).transpose(1, 0, 2)).astype(bf16)
        gi = structs["gidx"][k]
        gw = np.tile(np.ascontiguousarray(gi.reshape(-1, 16).T), (8, 1))
        in_maps.append({
            "xt": xs, "encw": encw, "w1": w1s, "w2": w2s, "w2r": w2r,
            "linw": linw65, "ident": ident, "params": params, "gidx": gw,
            "rmat": structs["rmat"][k],
        })
    return in_maps


def _run(inputs, cfg=CFG, trace=False, tmpdir=None):
    import sys
    sys.path.insert(0, "/root/problem")
    from concourse.bass_utils import run_bass_kernel_spmd

    structs = build_edge_structs(inputs["edge_index"], cfg)
    nc, NPCOL, PCOL = build_nc(structs, cfg)
    in_maps = pack_inputs(inputs, structs, NPCOL, PCOL, cfg)
    res = run_bass_kernel_spmd(nc, in_maps, list(range(NCORES)), trace=trace,
                               tmpdir=tmpdir)
    outs = [res.results[k]["outp"] for k in range(NCORES)]  # [C, NSH] each
    full = np.concatenate(outs, axis=1).T                   # [NPAD, C]
    return np.ascontiguousarray(full[: cfg.N]).astype(np.float32), res


def kernel(**inputs) -> np.ndarray:
    out, _ = _run(inputs)
    return out


# revision 33
# speedup vs baseline: 1.7311x; 1.7311x over previous
"""DeeperGCN Trainium2 kernel (8 NeuronCores, SPMD) — quarter-pipelined v2.

Strategy:
  - Nodes padded to NPAD = 8*NSH, sharded by dst across 8 cores; each
    core's nodes split into 4 quarters of NQ.
  - Per conv layer the per-node message tables u = m*exp(t*m), w =
    exp(t*m) (m = relu(h)+eps) are AllGathered in FOUR quarter pieces
    (one collective per (conv, quarter), double-buffered across convs)
    so edge gathers can start as soon as a piece lands.
  - Edge aggregation: per dst-quarter, 3-4 window groups; per group one
    dma_gather per src-quarter (4 SWDGE queues), indices sorted by
    source row within each (window, src-quarter) bucket for HBM
    locality, then segmented-sum over dst via TensorE matmuls with
    one-hot fp8 R matrices (host-precomputed, one streamed DMA/group).
  - The conv is pipelined at quarter granularity: after the groups of
    dst-quarter q finish, the MLP (M phase), next conv's U phase and
    its AllGather trigger for quarter q are interleaved with the
    gathers of quarter q+1, so collectives/MLP hide under gather DMA.
  - MLP+LN evaluated feature-major with LN mean-centering and biases
    folded into the weights host-side (ones-row trick), variance via
    ones-matmul of the square, rstd via ACT Abs_reciprocal_sqrt.
Host preprocessing (edge bucketing, R matrices, transpose/cast of x,
weight centering) is numpy; only device NEFF time counts.
"""
import numpy as np
import ml_dtypes
from dataclasses import dataclass

EPS_MSG = 1e-7
LN_EPS = 1e-5
NCORES = 8


@dataclass
class Cfg:
    N: int = 100000
    E: int = 1000000
    F_IN: int = 500
    H: int = 64
    C: int = 3
    L: int = 3
    NSH: int = 12800            # nodes/core, 100 windows of 128
    NSW: int = 4                # src quarters (one AllGather piece each)
    SLOTS_G: int = 104          # gather-buffer chunk slots per group
    MCH: int = 400              # MLP node-chunk (<=512)

    @property
    def NPAD(self):
        return NCORES * self.NSH

    @property
    def NW(self):
        return self.NSH // 128   # 100

    @property
    def NQ(self):
        return self.NSH // 4     # 3200

    @property
    def NWQ(self):
        return self.NW // 4      # 25 windows per quarter

    @property
    def WSZ(self):
        w = self.NPAD // self.NSW   # rows per src window (25600)
        assert w <= 32768
        return w

    @property
    def HH(self):
        return 2 * self.H

    @property
    def FPAD(self):
        return ((self.F_IN + 127) // 128) * 128

    @property
    def CPQ(self):
        return self.NQ // self.MCH   # 8


CFG = Cfg()


def _cdiv(a, b):
    return (a + b - 1) // b


# --------------------------------------------------------------------------
# host: edge structures
# --------------------------------------------------------------------------

def build_edge_structs(edge_index, cfg=CFG):
    src = np.asarray(edge_index[0], np.int64)
    dst = np.asarray(edge_index[1], np.int64)
    NSH, NW, NSW, NQ, NWQ = cfg.NSH, cfg.NW, cfg.NSW, cfg.NQ, cfg.NWQ

    WSZ = cfg.WSZ
    core = dst // NSH
    dstloc = dst % NSH
    w = dstloc // 128
    dcol = dstloc % 128
    s = src // WSZ                      # src window of the full table
    srow = src % WSZ                    # row within src window
    assert srow.max() < 32768

    counts = np.zeros((NCORES, NW, NSW), np.int64)
    np.add.at(counts, (core, w, s), 1)
    PBc = _cdiv(counts, 128).max(axis=0)           # [NW, NSW]
    PBc = np.maximum(PBc, 1)
    win_chunks = PBc.sum(axis=1)                   # [NW]

    # groups: per dst-quarter, balanced split with max group <= SLOTS_G
    groups = []                                    # (q, w0, w1)
    for q in range(4):
        wq = np.arange(q * NWQ, (q + 1) * NWQ)
        tot = int(win_chunks[wq].sum())
        ng = max(1, _cdiv(tot, cfg.SLOTS_G))
        while True:
            cuts = [q * NWQ]
            acc, target, gi = 0, tot / ng, 1
            ok = True
            sizes = []
            for wi in wq:
                acc += int(win_chunks[wi])
                if acc >= target * gi and gi < ng:
                    cuts.append(int(wi) + 1)
                    gi += 1
            cuts.append((q + 1) * NWQ)
            sizes = [int(win_chunks[cuts[i]:cuts[i + 1]].sum())
                     for i in range(len(cuts) - 1)]
            if max(sizes) <= cfg.SLOTS_G:
                break
            ng += 1
        for i in range(len(cuts) - 1):
            if cuts[i] < cuts[i + 1]:
                groups.append((q, cuts[i], cuts[i + 1]))

    # global slot layout: for si (gather phase), for group, for wi,
    # chunks contiguous -- phase si of all groups is one contiguous run
    sg_base = {}         # (gid, si) -> first global slot of that run
    wc_base = {}         # (wi, si) -> global slot base of that bucket
    slot = 0
    for si in range(NSW):
        for gid, (q, w0, w1) in enumerate(groups):
            sg_base[(gid, si)] = slot
            for wi in range(w0, w1):
                wc_base[(wi, si)] = slot
                slot += int(PBc[wi, si])
    CTOT = slot
    IWALL = CTOT * 128
    SLOTS_SI = max(int(PBc[w0:w1, si].sum())
                   for (q, w0, w1) in groups for si in range(NSW))

    # per-core gather indices + R matrices (src-sorted within buckets)
    gidx_all, rmat_all = [], []
    key = (w * NSW + s)
    for k in range(NCORES):
        sel = core == k
        kk, ksrow, kdcol = key[sel], srow[sel], dcol[sel]
        order = np.lexsort((ksrow, kk))
        kk, ksrow, kdcol = kk[order], ksrow[order], kdcol[order]
        bnd = np.searchsorted(kk, np.arange(NW * NSW + 1))
        gidx = np.zeros(IWALL, np.int16)
        rmat = np.zeros((128, CTOT, 128), ml_dtypes.float8_e4m3)
        for wi in range(NW):
            for si in range(NSW):
                a, b = bnd[wi * NSW + si], bnd[wi * NSW + si + 1]
                n = b - a
                cap = int(PBc[wi, si]) * 128
                assert n <= cap, (wi, si, n, cap)
                base = wc_base[(wi, si)] * 128
                gidx[base: base + n] = ksrow[a:b]
                j = np.arange(n)
                rmat[j % 128, wc_base[(wi, si)] + j // 128, kdcol[a:b]] = 1.0
        gidx_all.append(gidx)
        rmat_all.append(rmat)

    return dict(PBc=PBc, groups=groups, sg_base=sg_base,
                wc_base=wc_base, CTOT=CTOT, IWALL=IWALL, SLOTS_SI=SLOTS_SI,
                gidx=gidx_all, rmat=rmat_all)


# --------------------------------------------------------------------------
# device builder
# --------------------------------------------------------------------------

def build_nc(structs, cfg=CFG):
    import concourse.bass as bass
    import concourse.tile as tile
    from concourse import bacc, mybir
    from contextlib import ExitStack

    dt = mybir.dt
    AF = mybir.ActivationFunctionType
    AL = mybir.AluOpType
    PBc, groups = structs["PBc"], structs["groups"]
    sg_base = structs["sg_base"]
    wc_base, CTOT, IWALL = structs["wc_base"], structs["CTOT"], structs["IWALL"]
    NSH, NW, NSW, WSZ = cfg.NSH, cfg.NW, cfg.NSW, cfg.WSZ
    NQ, NWQ = cfg.NQ, cfg.NWQ
    H, HH, C, L, FPAD = cfg.H, cfg.HH, cfg.C, cfg.L, cfg.FPAD
    MCH, CPQ = cfg.MCH, cfg.CPQ
    FCH = FPAD // 128

    PCOL = {"encb": 0, "b2r": 1, "eps": 2, "tiny": 3, "zero": 4}
    nc_col = 5
    for l in range(L):
        for nm in ("g1", "be1", "b2c", "ng", "nb", "t"):
            PCOL[(nm, l)] = nc_col
            nc_col += 1
    NPCOL = nc_col

    nc = bacc.Bacc("TRN2", num_swdge_queues=4, dynamic_dma_scratch_size=32768)
    xt_d = nc.declare_dram_parameter("xt", [128, FCH, NSH], dt.bfloat16, isOutput=False)
    encw_d = nc.declare_dram_parameter("encw", [128, FCH, H], dt.bfloat16, isOutput=False)
    w1_d = nc.declare_dram_parameter("w1", [H + 1, L, HH], dt.bfloat16, isOutput=False)
    w2_d = nc.declare_dram_parameter("w2", [HH, L, H], dt.bfloat16, isOutput=False)
    w2r_d = nc.declare_dram_parameter("w2r", [HH, H], dt.bfloat16, isOutput=False)
    linw_d = nc.declare_dram_parameter("linw", [H + 1, C], dt.bfloat16, isOutput=False)
    ident_d = nc.declare_dram_parameter("ident", [128, 128], dt.bfloat16, isOutput=False)
    params_d = nc.declare_dram_parameter("params", [128, NPCOL], dt.float32, isOutput=False)
    gidx_d = nc.declare_dram_parameter("gidx", [128, IWALL // 16], dt.int16, isOutput=False)
    rmat_d = nc.declare_dram_parameter("rmat", [128, CTOT, 128], dt.float8e4, isOutput=False)
    outp_d = nc.declare_dram_parameter("outp", [C, NSH], dt.float32, isOutput=True)

    uvsh = [nc.dram_tensor(f"uvsh{p}", [NSH, HH], dt.bfloat16)
            for p in range(2)]
    tabs = [nc.dram_tensor(f"tab{p}", [cfg.NPAD, HH], dt.bfloat16,
                           addr_space="Shared") for p in range(2)]

    conv_params = [0] + list(range(L))          # [0, 0, 1, 2]

    with tile.TileContext(nc) as tc, ExitStack() as ctx:
        const = ctx.enter_context(tc.tile_pool(name="const", bufs=1))
        sb_par = const.tile([128, NPCOL], dt.float32)
        nc.sync.dma_start(sb_par[:], params_d[:])
        sb_encw = const.tile([128, FCH, H], dt.bfloat16)
        nc.sync.dma_start(sb_encw[:], encw_d[:])
        sb_w1 = const.tile([H + 1, L, HH], dt.bfloat16)
        nc.sync.dma_start(sb_w1[:], w1_d[:])
        sb_w2 = const.tile([HH, L, H], dt.bfloat16)
        nc.sync.dma_start(sb_w2[:], w2_d[:])
        sb_w2r = const.tile([HH, H], dt.bfloat16)
        nc.sync.dma_start(sb_w2r[:], w2r_d[:])
        sb_linw = const.tile([H + 1, C], dt.bfloat16)
        nc.sync.dma_start(sb_linw[:], linw_d[:])
        sb_id = const.tile([128, 128], dt.bfloat16)
        nc.sync.dma_start(sb_id[:], ident_d[:])
        sb_o128 = const.tile([128, 128], dt.bfloat16)
        nc.vector.memset(sb_o128[:], 1.0 / 128)
        sb_o64 = const.tile([H, H], dt.bfloat16)
        nc.vector.memset(sb_o64[:], 1.0 / H)

        def pcol(key, rows=128):
            cidx = PCOL[key]
            return sb_par[0:rows, cidx: cidx + 1]

        master = ctx.enter_context(tc.tile_pool(name="master", bufs=1))
        hT = master.tile([H + 1, NSH], dt.bfloat16)     # row H == 1.0
        rbf65 = master.tile([H + 1, NQ], dt.bfloat16)   # row H == 1.0
        nc.vector.memset(hT[H:H + 1, :], 1.0)
        nc.vector.memset(rbf65[H:H + 1, :], 1.0)

        # persistent pools
        ndq = ctx.enter_context(tc.tile_pool(name="ndq", bufs=4))
        uvq = ctx.enter_context(tc.tile_pool(name="uvq", bufs=1))
        utp = ctx.enter_context(tc.tile_pool(name="utp", bufs=1))
        stp = ctx.enter_context(tc.tile_pool(name="stp", bufs=2))
        t64 = ctx.enter_context(tc.tile_pool(name="t64", bufs=3))
        t128 = ctx.enter_context(tc.tile_pool(name="t128", bufs=3))
        otp = ctx.enter_context(tc.tile_pool(name="otp", bufs=2))
        tpp = ctx.enter_context(tc.tile_pool(name="tpp", bufs=1, space="PSUM"))

        # ------------- emit helpers -------------
        def emit_U(par, q, l):
            """uv table rows for quarter q from hT -> uvsh[par]."""
            slq = slice(q * NQ, (q + 1) * NQ)
            m_t = utp.tile([H, NQ], dt.bfloat16, tag="mt")
            nc.vector.tensor_scalar(m_t[:], hT[0:H, slq], 0.0, EPS_MSG,
                                    AL.max, AL.add)
            wq = utp.tile([H, NQ], dt.bfloat16, tag="wq")
            nc.scalar.activation(wq[:], m_t[:], AF.Exp, scale=pcol(("t", l), H))
            uv = uvq.tile([128, NQ], dt.bfloat16, tag="uv")
            nc.vector.tensor_scalar_add(uv[H:HH, :], wq[:], pcol("zero", H))
            nc.vector.tensor_mul(uv[0:H, :], m_t[:], wq[:])
            TB = 5
            for b in range(NWQ // TB):
                st = stp.tile([128, TB, HH], dt.bfloat16, tag="st")
                for t in range(TB):
                    nt = b * TB + t
                    tp = tpp.tile([128, HH], dt.bfloat16, tag="tp")
                    nc.tensor.transpose(tp[:], uv[:, nt * 128:(nt + 1) * 128],
                                        sb_id[:])
                    if nt % 2 == 0:
                        nc.scalar.copy(st[:, t, :], tp[:])
                    else:
                        nc.vector.tensor_copy(st[:, t, :], tp[:])
                r0 = q * NQ + b * TB * 128
                out_ap = uvsh[par][r0: r0 + TB * 128, :]
                nc.sync.dma_start(
                    out_ap.rearrange("(t p) h -> p t h", p=128), st[:])

        def emit_AG(par):
            nc.gpsimd.collective_compute(
                "AllGather", mybir.AluOpType.bypass,
                replica_groups=[list(range(NCORES))],
                ins=[uvsh[par][:, :]], outs=[tabs[par][:, :]])

        def emit_M(conv, l, q, nd):
            """agg -> MLP(+folded LN) -> residual for quarter q."""
            is_first = conv == 0
            slq = slice(q * NQ, (q + 1) * NQ)
            dn_lo = t64.tile([H, NQ], dt.bfloat16, tag="t64")
            nc.vector.tensor_scalar_max(dn_lo[:], nd[H:HH, :], 0.5)
            s_t = t64.tile([H, NQ], dt.bfloat16, tag="t64")
            nc.scalar.activation(s_t[:], dn_lo[:], AF.Abs_reciprocal_sqrt,
                                 bias=pcol("tiny", H))
            rden = t64.tile([H, NQ], dt.bfloat16, tag="t64")
            nc.vector.tensor_mul(rden[:], s_t[:], s_t[:])
            t1 = t64.tile([H, NQ], dt.bfloat16, tag="t64")
            nc.vector.tensor_mul(t1[:], nd[0:H, :], rden[:])
            nc.vector.tensor_add(rbf65[0:H, :], t1[:], hT[0:H, slq])
            ycs = t128.tile([HH, NQ], dt.bfloat16, tag="t128")
            rstd = t128.tile([HH, NQ], dt.bfloat16, tag="t128")
            for c in range(CPQ):
                lo = c * MCH
                ps1 = mpa.tile([HH, MCH], dt.float32, tag="mma")
                nc.tensor.matmul(ps1[:], sb_w1[:, l, :], rbf65[:, lo:lo + MCH])
                nc.vector.tensor_copy(ycs[:, lo:lo + MCH], ps1[:])
            sq = t128.tile([HH, NQ], dt.bfloat16, tag="t128")
            nc.vector.tensor_mul(sq[:], ycs[:], ycs[:])
            for c in range(CPQ):
                lo = c * MCH
                pv = mpa.tile([HH, MCH], dt.float32, tag="mma")
                nc.tensor.matmul(pv[:], sb_o128[:], sq[:, lo:lo + MCH])
                nc.scalar.activation(rstd[:, lo:lo + MCH], pv[:],
                                     AF.Abs_reciprocal_sqrt, bias=pcol("eps"))
            t_t = t128.tile([HH, NQ], dt.bfloat16, tag="t128")
            nc.vector.tensor_mul(t_t[:], ycs[:], rstd[:])
            h1 = t128.tile([HH, NQ], dt.bfloat16, tag="t128")
            nc.scalar.activation(h1[:], t_t[:], AF.Relu,
                                 bias=pcol(("be1", l)), scale=pcol(("g1", l)))
            if is_first:
                for c in range(CPQ):
                    lo = c * MCH
                    sl = slice(q * NQ + lo, q * NQ + lo + MCH)
                    ps2 = mpb.tile([H, MCH], dt.float32, tag="mmb")
                    nc.tensor.matmul(ps2[:], sb_w2r[:, :], h1[:, lo:lo + MCH])
                    nc.vector.tensor_scalar_add(hT[0:H, sl], ps2[:],
                                                pcol("b2r", H))
            else:
                ycs2 = t64.tile([H, NQ], dt.bfloat16, tag="t64")
                rstd2 = t64.tile([H, NQ], dt.bfloat16, tag="t64")
                for c in range(CPQ):
                    lo = c * MCH
                    ps2 = mpb.tile([H, MCH], dt.float32, tag="mmb")
                    nc.tensor.matmul(ps2[:], sb_w2[:, l, :], h1[:, lo:lo + MCH])
                    nc.vector.tensor_scalar_add(ycs2[:, lo:lo + MCH], ps2[:],
                                                pcol(("b2c", l), H))
                sq2 = t64.tile([H, NQ], dt.bfloat16, tag="t64")
                nc.vector.tensor_mul(sq2[:], ycs2[:], ycs2[:])
                for c in range(CPQ):
                    lo = c * MCH
                    pv2 = mpb.tile([H, MCH], dt.float32, tag="mmb")
                    nc.tensor.matmul(pv2[:], sb_o64[:], sq2[:, lo:lo + MCH])
                    nc.scalar.activation(rstd2[:, lo:lo + MCH], pv2[:],
                                         AF.Abs_reciprocal_sqrt,
                                         bias=pcol("eps", H))
                t2 = t64.tile([H, NQ], dt.bfloat16, tag="t64")
                nc.vector.tensor_mul(t2[:], ycs2[:], rstd2[:])
                c_t = t64.tile([H, NQ], dt.bfloat16, tag="t64")
                nc.scalar.activation(c_t[:], t2[:], AF.Relu,
                                     bias=pcol(("nb", l), H),
                                     scale=pcol(("ng", l), H))
                nc.vector.tensor_add(hT[0:H, slq], hT[0:H, slq], c_t[:])

        def emit_head(q):
            slq = slice(q * NQ, (q + 1) * NQ)
            yc = t128.tile([H, NQ], dt.bfloat16, tag="t128")
            rstd = t64.tile([H, NQ], dt.bfloat16, tag="t64")
            for c in range(CPQ):
                lo = c * MCH
                sl = slice(q * NQ + lo, q * NQ + lo + MCH)
                pmu = mpb.tile([H, MCH], dt.float32, tag="mmb")
                nc.tensor.matmul(pmu[:], sb_o64[:], hT[0:H, sl])
                nc.vector.tensor_sub(yc[:, lo:lo + MCH], hT[0:H, sl], pmu[:])
            sq = t64.tile([H, NQ], dt.bfloat16, tag="t64")
            nc.vector.tensor_mul(sq[:], yc[:], yc[:])
            for c in range(CPQ):
                lo = c * MCH
                pv = mpb.tile([H, MCH], dt.float32, tag="mmb")
                nc.tensor.matmul(pv[:], sb_o64[:], sq[:, lo:lo + MCH])
                nc.scalar.activation(rstd[:, lo:lo + MCH], pv[:],
                                     AF.Abs_reciprocal_sqrt, bias=pcol("eps", H))
            t_t = t64.tile([H, NQ], dt.bfloat16, tag="t64")
            nc.vector.tensor_mul(t_t[:], yc[:], rstd[:])
            f65 = t128.tile([H + 1, NQ], dt.bfloat16, tag="t128")
            nc.vector.memset(f65[H:H + 1, :], 1.0)
            nc.scalar.activation(f65[0:H, :], t_t[:], AF.Relu,
                                 bias=pcol(("nb", 0), H),
                                 scale=pcol(("ng", 0), H))
            for c in range(CPQ):
                lo = c * MCH
                sl = slice(q * NQ + lo, q * NQ + lo + MCH)
                pso = mpa.tile([C, MCH], dt.float32, tag="mma")
                nc.tensor.matmul(pso[:], sb_linw[:, :], f65[:, lo:lo + MCH])
                ot = otp.tile([C, MCH], dt.float32, tag="ot")
                nc.vector.tensor_copy(ot[:], pso[:])
                nc.sync.dma_start(outp_d[:, sl], ot[:])

        # ---------------- encoder (pipelined per quarter) ----------------
        with tc.tile_pool(name="enc", bufs=3) as ep, \
             tc.tile_pool(name="encps", bufs=2, space="PSUM") as pp:
            for q in range(4):
                for cc in range(CPQ):
                    sl = slice(q * NQ + cc * MCH, q * NQ + (cc + 1) * MCH)
                    xtile = ep.tile([128, FCH, MCH], dt.bfloat16)
                    nc.sync.dma_start(xtile[:], xt_d[:, :, sl])
                    ps = pp.tile([H, MCH], dt.float32)
                    for fc in range(FCH):
                        nc.tensor.matmul(ps[:], sb_encw[:, fc, :],
                                         xtile[:, fc, :],
                                         start=(fc == 0), stop=(fc == FCH - 1))
                    nc.vector.tensor_scalar_add(hT[0:H, sl], ps[:],
                                                pcol("encb", H))
                emit_U(0, q, conv_params[0])
            emit_AG(0)

        # gather/psum pools opened after encoder frees its SBUF
        gp = ctx.enter_context(tc.tile_pool(name="gp", bufs=8))
        rp = ctx.enter_context(tc.tile_pool(name="rp", bufs=8))
        gxp = ctx.enter_context(tc.tile_pool(name="gxp", bufs=8))
        gsp = ctx.enter_context(tc.tile_pool(name="gsp", bufs=3, space="PSUM"))
        mpa = ctx.enter_context(tc.tile_pool(name="mpa", bufs=2, space="PSUM"))
        mpb = ctx.enter_context(tc.tile_pool(name="mpb", bufs=2, space="PSUM"))

        # ---------------- conv layers ----------------
        # Baseline-proven phase ordering: a pure G+S phase (dense PE/Pool
        # streams, no MLP interleaved), then M for all quarters, then U,
        # then the single AllGather for the next conv.
        WT = 3      # windows per PSUM accumulation tile
        for conv, l in enumerate(conv_params):
            par = conv % 2
            ndt = {}
            cpy = 0
            for gid, (q, w0, w1) in enumerate(groups):
                if q not in ndt:
                    ndt[q] = ndq.tile([HH, NQ], dt.bfloat16, tag="nd",
                                      name=f"nd_{conv}_{q}")
                nd = ndt[q]
                gb, rt = [], []
                for si in range(NSW):
                    nsl = int(PBc[w0:w1, si].sum())
                    base = sg_base[(gid, si)]
                    rtile = rp.tile([128, nsl, 128], dt.float8e4, tag="rt",
                                    name=f"rt_{conv}_{gid}_{si}")
                    nc.sync.dma_start(rtile[:],
                                      rmat_d[:, base: base + nsl, :])
                    gx = gxp.tile([128, nsl * 8], dt.int16, tag="gx",
                                  name=f"gx_{conv}_{gid}_{si}")
                    nc.sync.dma_start(
                        gx[:], gidx_d[:, base * 8: (base + nsl) * 8])
                    gbuf = gp.tile([128, nsl, HH], dt.bfloat16, tag="gbuf",
                                   name=f"gb_{conv}_{gid}_{si}")
                    nc.gpsimd.dma_gather(
                        gbuf[:], tabs[par][si * WSZ: (si + 1) * WSZ, :],
                        gx[:], nsl * 128, nsl * 128, HH,
                        single_packet=False, queue_num=si)
                    gb.append(gbuf)
                    rt.append(rtile)
                wi = w0
                while wi < w1:
                    wn = min(WT, w1 - wi)
                    ps = gsp.tile([HH, WT * 128], dt.float32, tag="gs")
                    for k in range(wn):
                        nchw = int(PBc[wi + k].sum())
                        done = 0
                        for si in range(NSW):
                            cb = wc_base[(wi + k, si)] - sg_base[(gid, si)]
                            for j in range(int(PBc[wi + k, si])):
                                nc.tensor.matmul(
                                    ps[:, k * 128:(k + 1) * 128],
                                    gb[si][:, cb + j, :],
                                    rt[si][:, cb + j, :],
                                    start=(done == 0),
                                    stop=(done == nchw - 1))
                                done += 1
                    wloc = wi - q * NWQ
                    wsl = slice(wloc * 128, (wloc + wn) * 128)
                    if cpy % 2 == 0:
                        nc.scalar.copy(nd[:, wsl], ps[:, : wn * 128])
                    else:
                        nc.vector.tensor_copy(nd[:, wsl], ps[:, : wn * 128])
                    cpy += 1
                    wi += wn
            for q in range(4):
                emit_M(conv, l, q, ndt[q])
            if conv + 1 < len(conv_params):
                for q in range(4):
                    emit_U(1 - par, q, conv_params[conv + 1])
                emit_AG(1 - par)
            else:
                for q in range(4):
                    emit_head(q)

    nc.compile()
    return nc, NPCOL, PCOL


# --------------------------------------------------------------------------
# host: input packing
# --------------------------------------------------------------------------

def pack_inputs(inputs, structs, NPCOL, PCOL, cfg=CFG):
    bf16 = ml_dtypes.bfloat16
    NSH, NPAD, FPAD = cfg.NSH, cfg.NPAD, cfg.FPAD
    H, HH, C, L = cfg.H, cfg.HH, cfg.C, cfg.L

    x = np.asarray(inputs["x"], np.float32)
    xp = np.zeros((NPAD, FPAD), np.float32)
    xp[: x.shape[0], : x.shape[1]] = x

    encw = np.zeros((FPAD, H), np.float32)
    encw[: cfg.F_IN] = np.asarray(inputs["enc_W"], np.float32)
    encw = np.ascontiguousarray(
        encw.reshape(FPAD // 128, 128, H).transpose(1, 0, 2)).astype(bf16)

    # W1 with LN mean-centering folded in + centered bias as a 65th row
    W1 = np.asarray(inputs["W1"], np.float32)          # [L, H, HH]
    b1 = np.asarray(inputs["b1"], np.float32)          # [L, HH]
    W1c = W1 - W1.mean(axis=2, keepdims=True)
    b1c = b1 - b1.mean(axis=1, keepdims=True)
    w1s = np.concatenate([W1c, b1c[:, None, :]], axis=1)   # [L, H+1, HH]
    w1s = np.ascontiguousarray(w1s.transpose(1, 0, 2)).astype(bf16)

    W2 = np.asarray(inputs["W2"], np.float32)          # [L, HH, H]
    W2c = W2 - W2.mean(axis=2, keepdims=True)
    w2s = np.ascontiguousarray(W2c.transpose(1, 0, 2)).astype(bf16)
    w2r = np.ascontiguousarray(W2[0]).astype(bf16)     # uncentered, conv 0

    linw = np.asarray(inputs["lin_W"], np.float32)
    linb = np.asarray(inputs["lin_b"], np.float32)
    linw65 = np.concatenate([linw, linb[None, :]], axis=0).astype(bf16)
    ident = np.eye(128, dtype=bf16)

    b2 = np.asarray(inputs["b2"], np.float32)          # [L, H]
    b2c = b2 - b2.mean(axis=1, keepdims=True)

    params = np.zeros((128, NPCOL), np.float32)
    params[:H, PCOL["encb"]] = inputs["enc_b"]
    params[:H, PCOL["b2r"]] = b2[0]
    params[:, PCOL["eps"]] = LN_EPS
    params[:, PCOL["tiny"]] = 1e-30
    for l in range(L):
        params[:, PCOL[("g1", l)]] = inputs["g1"][l]
        params[:, PCOL[("be1", l)]] = inputs["be1"][l]
        params[:H, PCOL[("b2c", l)]] = b2c[l]
        params[:H, PCOL[("ng", l)]] = inputs["ng"][l]
        params[:H, PCOL[("nb", l)]] = inputs["nb"][l]
        params[:, PCOL[("t", l)]] = float(np.asarray(inputs["t"][l]))

    in_maps = []
    for k in range(NCORES):
        xs = np.ascontiguousarray(xp[k * NSH:(k + 1) * NSH].T)  # [FPAD, NSH]
        xs = np.ascontiguousarray(
            xs.reshape(FPAD // 128, 128, NSH).transpose(1, 0, 2)).astype(bf16)
        gi = structs["gidx"][k]
        gw = np.tile(np.ascontiguousarray(gi.reshape(-1, 16).T), (8, 1))
        in_maps.append({
            "xt": xs, "encw": encw, "w1": w1s, "w2": w2s, "w2r": w2r,
            "linw": linw65, "ident": ident, "params": params, "gidx": gw,
            "rmat": structs["rmat"][k],
        })
    return in_maps


def _run(inputs, cfg=CFG, trace=False, tmpdir=None):
    import sys
    sys.path.insert(0, "/root/problem")
    from concourse.bass_utils import run_bass_kernel_spmd

    structs = build_edge_structs(inputs["edge_index"], cfg)
    nc, NPCOL, PCOL = build_nc(structs, cfg)
    in_maps = pack_inputs(inputs, structs, NPCOL, PCOL, cfg)
    res = run_bass_kernel_spmd(nc, in_maps, list(range(NCORES)), trace=trace,
                               tmpdir=tmpdir)
    outs = [res.results[k]["outp"] for k in range(NCORES)]  # [C, NSH] each
    full = np.concatenate(outs, axis=1).T                   # [NPAD, C]
    return np.ascontiguousarray(full[: cfg.N]).astype(np.float32), res


def kernel(**inputs) -> np.ndarray:
    out, _ = _run(inputs)
    return out
